# revision 1
# baseline (speedup 1.0000x reference)
"""Trainium2 Bass kernel for nn_LongTermAttention (continuous-basis long-term attention).

Strategy: pure data-parallel over batch (B=8 -> one batch element per NeuronCore).
Per core, the computation is restructured so the full [q, NB] score matrix is
never materialized:

  maskT[l, h]  = sigmoid(W_mask.T(stationary) contracted with k) + b_mask      (PE + ACT)
  kmT[l, h]    = k * maskT                                                     (DVE)
  BmatT[j, n~] = kmT.T @ Gs_perm   (n~ = sigma-deinterleaved basis order)      (PE)
  v_mu/v_sig   = kmT.T @ (Gs @ w_mu / w_sigma)   (host-folded into Gs_aug)     (PE)
  vals[n~, j]  = BmatT.T @ Wv.T                                                (PE)
  u            = [v_mu; v_sig] @ Wk.T / sqrt(d)                                (PE)
  Wtilde       = per-head block-diag expansion of u, contracted with Wq        (PE)
  mu_pre[32,q] = Wtilde.T @ qT  (rows 0-15: mu channel, 16-31: sigma channel)  (PE)
  grids        = sigmoid/softplus/recip/ln -> per-(s,h,q) quadratic coeffs     (ACT/DVE)
  g[n~, q]     = K=3 matmul: [lin^2; lin; 1].T @ [q1; q2; q3]  (the Gaussian
                 exponent incl. normalization), r = Exp(g) on PSUM eviction    (PE + ACT)
  ctx.T[d, q]  = vals_h.T @ r_h  (2 heads per PSUM tile via column tiling)     (PE)
  out[q, o]    = ctx.T.T @ Wo.T                                                (PE)

All matmuls run as float32r (fast fp32 path). Weights are pre-transposed and
basis-derived constants are precomputed on the host as part of input layout.
"""
import os
from contextlib import ExitStack

import numpy as np

import concourse.bass as bass
import concourse.tile as tile
from concourse import bacc, mybir
from concourse.bass_utils import run_bass_kernel_spmd
from concourse.masks import make_identity

F32 = mybir.dt.float32
F32R = mybir.dt.float32r
AF = mybir.ActivationFunctionType
AL = mybir.AluOpType

L = 2048          # memory length
NB = 512          # num basis
NB2 = 256         # per-sigma basis count
HID = 1024
H = 16
D = 64
B = 8
Q = 2048
LT = L // 128     # 16
JT = HID // 128   # 8
QTI = Q // 128    # 16
SIGMAS = (0.005, 0.01)
TWO_PI = 6.283185307179586


def build_nc():
    nc = bacc.Bacc("TRN2", target_bir_lowering=False, debug=False)

    k_d = nc.dram_tensor("k", [L, HID], F32, kind="ExternalInput").ap()
    qt_d = nc.dram_tensor("qt", [HID, Q], F32, kind="ExternalInput").ap()
    wm_d = nc.dram_tensor("wmT", [L, L], F32, kind="ExternalInput").ap()
    gs_d = nc.dram_tensor("gs_aug", [L, NB + 2], F32, kind="ExternalInput").ap()
    wv_d = nc.dram_tensor("wvT", [HID, HID], F32, kind="ExternalInput").ap()
    wk_d = nc.dram_tensor("wkT", [HID, HID], F32, kind="ExternalInput").ap()
    wq_d = nc.dram_tensor("wq", [HID, HID], F32, kind="ExternalInput").ap()
    wo_d = nc.dram_tensor("woT", [HID, HID], F32, kind="ExternalInput").ap()
    pb_d = nc.dram_tensor("p_basis", [3, NB2], F32, kind="ExternalInput").ap()
    bm_d = nc.dram_tensor("bm2d", [128, LT], F32, kind="ExternalInput").ap()
    out_d = nc.dram_tensor("out", [Q, HID], F32, kind="ExternalOutput").ap()

    with tile.TileContext(nc) as tc:
        pools = []

        def P(name, **kw):
            p = tc.alloc_tile_pool(name=name, bufs=kw.pop("bufs", 1), **kw)
            pools.append(p)
            return p  # NOTE: pools must be released in LIFO order per side

        def rel(*ps):
            for p in ps:
                p.release()
                pools.remove(p)

        cpool = P("cpool")
        bm_sb = cpool.tile([128, LT], F32, name="bm_sb")
        nc.sync.dma_start(bm_sb[:], bm_d)
        p5 = cpool.tile([5, NB2], F32R, name="p5")
        id2 = cpool.tile([2, 2], F32, name="id2")
        make_identity(nc, id2)
        id32 = cpool.tile([32, 32], F32, name="id32")
        make_identity(nc, id32)
        zt = cpool.tile([128, 1], F32, name="zt")
        nc.vector.memset(zt[:], 0.0)

        # ---------------- Phase 2 allocs (early, overlap with phase 1) ---------
        NBA = NB + 2  # 514
        bmP = P("bmP", side="right")
        bmT = bmP.tile([128, JT * NBA], F32, name="bmT")
        gs_all = bmP.tile([128, LT * NBA], F32, name="gs_all")
        nc.sync.dma_start(gs_all.rearrange("p (t c) -> p t c", t=LT),
                          gs_d.rearrange("(t p) c -> p t c", p=128))
        # ---------------- Phase 1: mask matmul + gated keys (kmT) -------------
        kmP = P("kmP")
        kmT = kmP.tile([128, LT * HID], F32, name="kmT")

        ph1 = P("ph1", bufs=1)
        ps1 = P("ps1", space="PSUM")
        p_sb = ph1.tile([3, NB2], F32, name="p_sb")
        nc.sync.dma_start(p_sb[:], pb_d)
        ph3 = ph1.tile([3, NB2], F32R, name="ph3")
        nc.vector.tensor_copy(ph3[:], p_sb[:])
        pl3 = ph1.tile([3, NB2], F32R, name="pl3")
        nc.vector.tensor_tensor(pl3[:], p_sb[:], ph3[:], AL.subtract)
        nc.sync.dma_start(p5[0:1, :], ph3[0:1, :])
        nc.sync.dma_start(p5[1:2, :], pl3[0:1, :])
        nc.sync.dma_start(p5[2:3, :], ph3[1:2, :])
        nc.sync.dma_start(p5[3:4, :], pl3[1:2, :])
        nc.sync.dma_start(p5[4:5, :], ph3[2:3, :])
        k_all = ph1.tile([128, LT * HID], F32R, name="k_all")
        for kc in range(4):
            nc.sync.dma_start(
                k_all[:, kc * 4 * HID:(kc + 1) * 4 * HID]
                .rearrange("p (t h) -> p t h", t=4),
                k_d[kc * 512:(kc + 1) * 512, :]
                .rearrange("(t p) h -> p t h", p=128).bitcast(F32R))
        for mt in range(LT):
            wm_t = ph1.tile([128, L], F32R, name="wm_t", tag="wm", bufs=2)
            nc.sync.dma_start(
                wm_t.rearrange("p (t c) -> p t c", t=LT),
                wm_d[:, mt * 128:(mt + 1) * 128]
                .rearrange("(t p) c -> p t c", p=128).bitcast(F32R))
            mp = ps1.tile([128, HID], F32, name="mp", tag="mp", bufs=2)
            for lt in range(LT):
                for nch in range(2):
                    nc.tensor.matmul(
                        mp[:, nch * 512:(nch + 1) * 512],
                        wm_t[:, lt * 128:(lt + 1) * 128],
                        k_all[:, lt * HID + nch * 512: lt * HID + nch * 512 + 512],
                        start=(lt == 0), stop=(lt == LT - 1))
            sg = ph1.tile([128, HID], F32, name="sg", tag="sg", bufs=2)
            nc.scalar.activation(sg[:], mp[:], AF.Sigmoid, bias=bm_sb[:, mt:mt + 1])
            nc.vector.tensor_tensor(
                kmT[:, mt * HID:(mt + 1) * HID],
                k_all[:, mt * HID:(mt + 1) * HID], sg[:], AL.mult)
        rel(ps1, ph1)

        wfull = P("wfull", side="right")
        ps2 = P("ps2", space="PSUM")
        for jt in range(JT):
            bp = ps2.tile([128, NBA], F32, name="bp", tag="bp", bufs=2)
            for lt in range(LT):
                lhsT = kmT[:, lt * HID + jt * 128: lt * HID + jt * 128 + 128]
                nc.tensor.matmul(bp[:, 0:512], lhsT,
                                 gs_all[:, lt * NBA: lt * NBA + 512],
                                 start=(lt == 0), stop=(lt == LT - 1))
                nc.tensor.matmul(bp[:, 512:514], lhsT,
                                 gs_all[:, lt * NBA + 512: lt * NBA + 514],
                                 start=(lt == 0), stop=(lt == LT - 1))
            nc.vector.tensor_copy(bmT[:, jt * NBA:(jt + 1) * NBA], bp[:])
        rel(ps2, kmP)

        # ---------------- Phase 3: vals, u, Wtilde ---------------------------
        valsP = P("valsP")
        vals_all = valsP.tile([128, 4 * HID], F32, name="vals_all")
        sm = P("sm")
        u_sb = sm.tile([2, HID], F32, name="u_sb")
        ubar = sm.tile([128, JT * 32], F32R, name="ubar")
        wtT = sm.tile([32, HID], F32, name="wtT")
        wt_all = sm.tile([128, JT * 32], F32R, name="wt_all")

        ps3a = P("ps3a", space="PSUM")
        vps = [ps3a.tile([128, HID], F32, name=f"vp{nt}", tag="vp", bufs=4)
               for nt in range(4)]
        for half in range(2):
            wvh = wfull.tile([128, 4 * HID], F32, name="wvh", tag="wf", bufs=2)
            nc.sync.dma_start(
                wvh.rearrange("p (t c) -> p t c", t=4),
                wv_d[half * 512:(half + 1) * 512, :]
                .rearrange("(t p) c -> p t c", p=128))
            for nt in range(4):
                for jt2 in range(4):
                    jt = half * 4 + jt2
                    for nch in range(2):
                        nc.tensor.matmul(
                            vps[nt][:, nch * 512:(nch + 1) * 512],
                            bmT[:, jt * NBA + nt * 128: jt * NBA + nt * 128 + 128],
                            wvh[:, jt2 * HID + nch * 512: jt2 * HID + nch * 512 + 512],
                            start=(jt == 0), stop=(jt == JT - 1))
        for nt in range(4):
            nc.vector.tensor_copy(vals_all[:, nt * HID:(nt + 1) * HID], vps[nt][:])
        rel(ps3a)

        ps3b = P("ps3b", space="PSUM")
        up = ps3b.tile([2, HID], F32, name="up", tag="up")
        for half in range(2):
            wkh = wfull.tile([128, 4 * HID], F32, name="wkh", tag="wf", bufs=2)
            nc.sync.dma_start(
                wkh.rearrange("p (t c) -> p t c", t=4),
                wk_d[half * 512:(half + 1) * 512, :]
                .rearrange("(t p) c -> p t c", p=128))
            for jt2 in range(4):
                jt = half * 4 + jt2
                for nch in range(2):
                    nc.tensor.matmul(
                        up[:, nch * 512:(nch + 1) * 512],
                        bmT[:, jt * NBA + 512: jt * NBA + 514],
                        wkh[:, jt2 * HID + nch * 512: jt2 * HID + nch * 512 + 512],
                        start=(jt == 0), stop=(jt == JT - 1))
        nc.scalar.mul(u_sb[:], up[:], 1.0 / (D ** 0.5))

        nc.vector.tensor_copy(ubar[:], zt[:, 0:1].to_broadcast((128, JT * 32)))
        for c in range(JT):
            tp = ps3b.tile([128, 2], F32, name="tp", tag="tp", bufs=2)
            nc.tensor.transpose(tp[:], u_sb[:, c * 128:(c + 1) * 128], id2[:])
            base = c * 32
            nc.vector.tensor_copy(ubar[0:64, base + 2 * c: base + 2 * c + 1],
                                  tp[0:64, 0:1])
            nc.vector.tensor_copy(ubar[64:128, base + 2 * c + 1: base + 2 * c + 2],
                                  tp[64:128, 0:1])
            nc.vector.tensor_copy(ubar[0:64, base + 16 + 2 * c: base + 16 + 2 * c + 1],
                                  tp[0:64, 1:2])
            nc.vector.tensor_copy(ubar[64:128, base + 17 + 2 * c: base + 18 + 2 * c],
                                  tp[64:128, 1:2])

        wtp = ps3b.tile([32, HID], F32, name="wtp", tag="wtp")
        for half in range(2):
            wqh = wfull.tile([128, 4 * HID], F32R, name="wqh", tag="wf", bufs=2)
            nc.sync.dma_start(
                wqh.rearrange("p (t c) -> p t c", t=4),
                wq_d[half * 512:(half + 1) * 512, :]
                .rearrange("(t p) c -> p t c", p=128).bitcast(F32R))
            for c2 in range(4):
                c = half * 4 + c2
                for nch in range(2):
                    nc.tensor.matmul(
                        wtp[:, nch * 512:(nch + 1) * 512],
                        ubar[:, c * 32:(c + 1) * 32],
                        wqh[:, c2 * HID + nch * 512: c2 * HID + nch * 512 + 512],
                        start=(c == 0), stop=(c == JT - 1))
        nc.scalar.copy(wtT[:], wtp[:])
        for c in range(JT):
            tp2 = ps3b.tile([128, 32], F32, name="tp2", tag="tp2", bufs=2)
            nc.tensor.transpose(tp2[:], wtT[:, c * 128:(c + 1) * 128], id32[:])
            nc.vector.tensor_copy(wt_all[:, c * 32:(c + 1) * 32], tp2[:])
        rel(ps3b, wfull, bmP)

        # ---------------- Phase 4: mu_pre ------------------------------------
        t16P = P("t16P", side="right")
        t16 = t16P.tile([16, 2 * Q], F32, name="t16")
        ph4 = P("ph4")
        ps4 = P("ps4", space="PSUM")
        qt_all = ph4.tile([128, JT * Q], F32R, name="qt_all")
        nc.sync.dma_start(qt_all.rearrange("p (t c) -> p t c", t=JT),
                          qt_d.rearrange("(t p) c -> p t c", p=128).bitcast(F32R))
        mupA = ps4.tile([16, Q], F32, name="mupA", tag="mupA")
        mupB = ps4.tile([16, Q], F32, name="mupB", tag="mupB")
        for kt in range(JT):
            for qc in range(4):
                rhs = qt_all[:, kt * Q + qc * 512: kt * Q + qc * 512 + 512]
                nc.tensor.matmul(mupA[:, qc * 512:(qc + 1) * 512],
                                 wt_all[:, kt * 32: kt * 32 + 16], rhs,
                                 start=(kt == 0), stop=(kt == JT - 1))
                nc.tensor.matmul(mupB[:, qc * 512:(qc + 1) * 512],
                                 wt_all[:, kt * 32 + 16: kt * 32 + 32], rhs,
                                 start=(kt == 0), stop=(kt == JT - 1))
        nc.scalar.copy(t16[:, 0:Q], mupA[:])
        nc.scalar.copy(t16[:, Q:2 * Q], mupB[:])
        rel(ps4, ph4, sm)

        # ---------------- Phase 5: per-(s,h,q) quadratic coefficient grids ----
        woP = P("woP")
        wo = woP.tile([128, JT * HID], F32R, name="wo")
        nc.sync.dma_start(wo.rearrange("p (t c) -> p t c", t=JT),
                          wo_d.rearrange("(t p) c -> p t c", p=128).bitcast(F32R))
        qgP = P("qgP")
        gq1 = [qgP.tile([16, Q], F32R, name=f"gq1_{s}") for s in range(2)]
        gq2 = [qgP.tile([16, Q], F32R, name=f"gq2_{s}") for s in range(2)]
        gq3 = [qgP.tile([16, Q], F32R, name=f"gq3_{s}") for s in range(2)]
        gt = P("gt", side="right")
        gmu = gt.tile([16, Q], F32, name="gmu")
        gsp = gt.tile([16, Q], F32, name="gsp")
        gss = gt.tile([16, Q], F32, name="gss")
        gvs = gt.tile([16, Q], F32, name="gvs", tag="gvs", bufs=1)
        givr = gt.tile([16, Q], F32, name="givr", tag="givr", bufs=1)
        gscr = gt.tile([16, Q], F32, name="gscr", tag="gscr", bufs=1)
        gln = gt.tile([16, Q], F32, name="gln", tag="gln", bufs=1)

        nc.scalar.activation(gmu[:], t16[:, 0:Q], AF.Sigmoid)
        # softplus(x) = ln(exp(x) + 1); input range is ~[-1, 1] so exp is safe
        nc.scalar.activation(gsp[:], t16[:, Q:2 * Q], AF.Exp)
        nc.scalar.activation(gss[:], gsp[:], AF.Ln, bias=1.0)
        nc.vector.tensor_scalar_max(gss[:], gss[:], 1e-6)
        for s in range(2):
            if s > 0:
                gvs = gt.tile([16, Q], F32, name="gvs", tag="gvs", bufs=1)
                givr = gt.tile([16, Q], F32, name="givr", tag="givr", bufs=1)
                gscr = gt.tile([16, Q], F32, name="gscr", tag="gscr", bufs=1)
                gln = gt.tile([16, Q], F32, name="gln", tag="gln", bufs=1)
            nc.vector.tensor_scalar_add(gvs[:], gss[:], SIGMAS[s] ** 2)
            nc.vector.reciprocal_approx_accurate(givr[:], gvs[:], gscr[:])
            nc.scalar.activation(gln[:], gvs[:], AF.Ln, scale=TWO_PI)
            nc.vector.tensor_scalar_mul(gq1[s][:], givr[:], -0.5)
            # q2 = (-2*mu)*q1 = iv*mu ; t3 = (-0.5*mu)*q2 = -0.5*iv*mu^2
            nc.vector.scalar_tensor_tensor(gq2[s][:], gmu[:], -2.0, gq1[s][:],
                                           AL.mult, AL.mult)
            nc.vector.scalar_tensor_tensor(gscr[:], gmu[:], -0.5, gq2[s][:],
                                           AL.mult, AL.mult)
            nc.vector.scalar_tensor_tensor(gq3[s][:], gln[:], -0.5, gscr[:],
                                           AL.mult, AL.add)
        rel(gt, t16P)

        # ---------------- Phase 6: r = exp(g) and context ---------------------
        ctxP = P("ctxP", side="right")
        ctxT = ctxP.tile([128, 8 * Q], F32R, name="ctxT")
        qp = P("qp")
        rp = P("rp")
        tmpP = P("tmpP")
        ps6 = P("ps6", space="PSUM")
        for h in range(H):
            p, odd = divmod(h, 2)
            cxp = ps6.tile([64, Q], F32, name="cxp", tag="cxp", bufs=1)
            for s in range(2):
                qt_t = qp.tile([5, Q], F32R, name="qt_t", tag="qt", bufs=2)
                nc.sync.dma_start(qt_t[0:1, :], gq1[s][h:h + 1, :])
                nc.sync.dma_start(qt_t[1:2, :], gq1[s][h:h + 1, :])
                nc.sync.dma_start(qt_t[2:3, :], gq2[s][h:h + 1, :])
                nc.sync.dma_start(qt_t[3:4, :], gq2[s][h:h + 1, :])
                nc.sync.dma_start(qt_t[4:5, :], gq3[s][h:h + 1, :])
                for t in range(2):
                    nt = 2 * s + t
                    for qh in range(2):
                        gp = ps6.tile([128, 1024], F32, name="gp", tag="gp",
                                      bufs=2)
                        for cc in range(2):
                            nc.tensor.matmul(
                                gp[:, cc * 512:(cc + 1) * 512],
                                p5[:, t * 128:(t + 1) * 128],
                                qt_t[:, qh * 1024 + cc * 512:
                                     qh * 1024 + cc * 512 + 512],
                                start=True, stop=True)
                        rt = rp.tile([128, 1024], F32, name="rt", tag="rt",
                                     bufs=3)
                        nc.scalar.activation(rt[:], gp[:], AF.Exp)
                        for cc in range(2):
                            qc = qh * 2 + cc
                            nc.tensor.matmul(
                                cxp[:, qc * 512:(qc + 1) * 512],
                                vals_all[:, nt * HID + h * D:
                                         nt * HID + h * D + D],
                                rt[:, cc * 512:(cc + 1) * 512],
                                start=(s == 0 and t == 0),
                                stop=(s == 1 and t == 1),
                                skip_group_check=True)
            if not odd:
                nc.vector.tensor_copy(ctxT[0:64, p * Q:(p + 1) * Q], cxp[:])
            else:
                t64 = tmpP.tile([64, Q], F32R, name="t64", tag="t64", bufs=2)
                nc.vector.tensor_copy(t64[:], cxp[:])
                nc.sync.dma_start(ctxT[64:128, p * Q:(p + 1) * Q], t64[:])
        rel(ps6, tmpP, rp, qp, qgP)

        # ---------------- Phase 7: output projection --------------------------
        outP = P("outP")
        ps7 = P("ps7", space="PSUM")
        for qi in range(QTI):
            op = ps7.tile([128, HID], F32, name="op", tag="op", bufs=2)
            for jt in range(JT):
                for och in range(2):
                    nc.tensor.matmul(
                        op[:, och * 512:(och + 1) * 512],
                        ctxT[:, jt * Q + qi * 128: jt * Q + qi * 128 + 128],
                        wo[:, jt * HID + och * 512: jt * HID + och * 512 + 512],
                        start=(jt == 0), stop=(jt == JT - 1))
            ob = outP.tile([128, HID], F32, name="ob", tag="ob", bufs=2)
            nc.vector.tensor_copy(ob[:], op[:])
            nc.sync.dma_start(out_d[qi * 128:(qi + 1) * 128, :], ob[:])
        rel(ps7, outP, ctxP, woP, valsP, cpool)

    nc.compile()
    return nc


def _host_prep(W_mask, Wq, Wk, Wv, Wo, w_mu, w_sigma, Gs, b_mask):
    Gs = np.asarray(Gs, np.float32)
    perm = np.concatenate([np.arange(0, NB, 2), np.arange(1, NB, 2)])
    gs_aug = np.concatenate(
        [Gs[:, perm],
         (Gs @ np.asarray(w_mu, np.float32))[:, None],
         (Gs @ np.asarray(w_sigma, np.float32))[:, None]], axis=1)
    gs_aug = np.ascontiguousarray(gs_aug, np.float32)
    lin = np.linspace(0.0, 1.0, NB2, dtype=np.float64)
    p_basis = np.stack([lin * lin, lin, np.ones_like(lin)]).astype(np.float32)
    bm2d = np.ascontiguousarray(
        np.asarray(b_mask, np.float32).reshape(LT, 128).T)
    return {
        "wmT": np.ascontiguousarray(np.asarray(W_mask, np.float32).T),
        "gs_aug": gs_aug,
        "wvT": np.ascontiguousarray(np.asarray(Wv, np.float32).T),
        "wkT": np.ascontiguousarray(np.asarray(Wk, np.float32).T),
        "wq": np.ascontiguousarray(np.asarray(Wq, np.float32)),
        "woT": np.ascontiguousarray(np.asarray(Wo, np.float32).T),
        "p_basis": p_basis,
        "bm2d": bm2d,
    }


_NC_CACHE = {}


def _get_nc():
    if "nc" not in _NC_CACHE:
        _NC_CACHE["nc"] = build_nc()
    return _NC_CACHE["nc"]


def kernel(k, query, W_mask, b_mask, Wq, Wk, Wv, Wo, w_mu, w_sigma,
           Gs, basis_mu, basis_sigma, _trace=False):
    k = np.asarray(k, np.float32)
    query = np.asarray(query, np.float32)
    shared = _host_prep(W_mask, Wq, Wk, Wv, Wo, w_mu, w_sigma, Gs, b_mask)
    in_maps = []
    for b in range(B):
        m = dict(shared)
        m["k"] = np.ascontiguousarray(k[b])
        m["qt"] = np.ascontiguousarray(
            query[b].transpose(0, 2, 1).reshape(HID, Q))
        in_maps.append(m)
    nc = _get_nc()
    res = run_bass_kernel_spmd(nc, in_maps, core_ids=list(range(B)),
                               trace=_trace)
    out = np.stack([res.results[b]["out"] for b in range(B)])
    if _trace:
        return out, res
    return out

